# revision 1
# baseline (speedup 1.0000x reference)
"""Paged-attention decode kernel for 8 TRN2 NeuronCores (Bass/Tile).

Problem: nn_Attention_15229954031958 (sparse_attention, memory-bound).
  q [32, 32, 128] f32, k/v_cache [8192, 16, 8, 128] f32,
  block_tables [32, 256] i32, context_lens [32] i32 -> out [32, 32, 128] f32.

Sharding: tensor-parallel over KV heads. Core c holds the head-c slice of
both caches (cast to bf16) plus q heads 4c..4c+3; no cross-core
communication. Every core runs the same graph (trip counts baked from
context_lens, identical on all cores), so SPMD is trivially satisfied.

Device-side dataflow per sequence (nb = ceil(ctx/16) pages):
  - dma_gather(transpose=True) pulls the nb K pages straight into
    K^T layout [d=128, t=16, page] (bf16, 16-bit xbar transpose).
  - dma_gather(transpose=False) pulls V pages into [page, (t, d)].
  - per 128-token chunk (page-group g, token-slot t):
      scores[s,4] = ktile[:, t, g*128:...]^T @ qT[:, 4b:4b+4]  (PSUM)
  - one batched exp per page-group on ScalarE: [128, 64] PSUM -> bf16 p.
  - PV: out[d,4] += vtile[0:jt, g, t*128:...]^T @ p[0:jt, 4t:4t+4]
    accumulated in PSUM over all chunks; masking is done by the static
    contraction subrange jt (tokens beyond context_len are never read).
  - denominators: one matmul per page-group (p^T @ ones -> [64,1] PSUM,
    accumulated), collapsed to [4,1] with a selector matmul at the end.
  - epilogue: PE-transpose out to [4,128], multiply by 1/den, and DMA the
    [4,128] result slice per sequence.
"""

import os
import numpy as np
import ml_dtypes

BLOCK = 16
D = 128
B = 32
H = 32
KVH = 8
G = 4
NBLK = 8192
MAXB = 256
SCALE = 0.08838834764831845
BF16 = ml_dtypes.bfloat16

_GRAPH_CACHE = {}


def _round_up(x, m):
    return (x + m - 1) // m * m


def _build_graph(ctx_lens, repeat=1, bufs=(3, 3, 3, 2, 2, 2), mode="full"):
    """Build + compile the SPMD graph for the given context lengths.

    repeat > 1 duplicates the whole body (for timing: slope difference
    between repeat=R and repeat=1 isolates pure HW time).
    mode: "full" | "dma" (gathers only) | "compute" (no gathers)."""
    import concourse.bass as bass
    import concourse.tile as tile
    from concourse import bacc, mybir
    from concourse.masks import make_identity
    from contextlib import ExitStack

    kb, vb, pb, eb, sb, ob = bufs
    nbs = [max(1, -(-int(c) // BLOCK)) for c in ctx_lens]
    nks = [_round_up(nb, 128) for nb in nbs]
    idx_cols = sum(nk // 16 for nk in nks)
    # process sequences largest-first: best prefetch ramp at the start and a
    # minimal non-overlapped tail (last gather is the smallest sequence)
    order = list(np.argsort(-np.asarray(nbs), kind="stable"))

    nc = bacc.Bacc("TRN2", target_bir_lowering=False, debug=False)

    k_src = nc.dram_tensor("k_src", [NBLK, BLOCK * D], mybir.dt.bfloat16,
                           kind="ExternalInput").ap()
    v_src = nc.dram_tensor("v_src", [NBLK, BLOCK * D], mybir.dt.bfloat16,
                           kind="ExternalInput").ap()
    qT_d = nc.dram_tensor("qT", [D, B * G], mybir.dt.float32,
                          kind="ExternalInput").ap()
    idx_d = nc.dram_tensor("idx", [128, idx_cols], mybir.dt.int16,
                           kind="ExternalInput").ap()
    sel_d = nc.dram_tensor("sel", [16 * G, G], mybir.dt.float32,
                           kind="ExternalInput").ap()
    mask_d = nc.dram_tensor("mask", [B, 128, 16 * G], mybir.dt.int8,
                            kind="ExternalInput").ap()
    out_d = nc.dram_tensor("out", [B, G, D], mybir.dt.float32,
                           kind="ExternalOutput").ap()

    with tile.TileContext(nc) as tc, ExitStack() as ctx:
        const = ctx.enter_context(tc.tile_pool(name="const", bufs=1))
        kpool = ctx.enter_context(tc.tile_pool(name="kpool", bufs=kb))
        vpool = ctx.enter_context(tc.tile_pool(name="vpool", bufs=vb))
        ppool = ctx.enter_context(tc.tile_pool(name="ppool", bufs=pb))
        epool = ctx.enter_context(tc.tile_pool(name="epool", bufs=eb))
        spsum = ctx.enter_context(tc.tile_pool(name="spsum", bufs=sb, space="PSUM"))
        opsum = ctx.enter_context(tc.tile_pool(name="opsum", bufs=ob, space="PSUM"))
        dpsum = ctx.enter_context(tc.tile_pool(name="dpsum", bufs=1, space="PSUM"))
        tpsum = ctx.enter_context(tc.tile_pool(name="tpsum", bufs=1, space="PSUM"))
        npsum = ctx.enter_context(tc.tile_pool(name="npsum", bufs=1, space="PSUM"))

        qT_sb = const.tile([128, B * G], mybir.dt.float32)
        nc.sync.dma_start(out=qT_sb[:], in_=qT_d[:])
        qT_b = const.tile([128, B * G], mybir.dt.bfloat16)
        nc.vector.tensor_copy(out=qT_b[:], in_=qT_sb[:])

        ident = const.tile([128, 128], mybir.dt.float32)
        make_identity(nc, ident[:])

        ones_b = const.tile([128, 1], mybir.dt.bfloat16)
        nc.vector.memset(ones_b[:], 1.0)

        sel_sb = const.tile([16 * G, G], mybir.dt.float32)
        nc.sync.dma_start(out=sel_sb[:], in_=sel_d[:])

        idx_sb = const.tile([128, idx_cols], mybir.dt.int16)
        nc.sync.dma_start(out=idx_sb[:], in_=idx_d[:])

        for _rep in range(repeat):
          col = 0
          for bi, b in enumerate(order):
            ctx_b = int(ctx_lens[b])
            nb, nk = nbs[b], nks[b]
            ng = nk // 128
            fb = ctx_b // BLOCK       # full pages
            rem = ctx_b % BLOCK       # tokens in the partial page

            ktile = kpool.tile([128, 16 * nk], mybir.dt.bfloat16, tag="kt")
            kap = ktile[:].rearrange("p (t n) -> p t n", t=16)
            vtile = vpool.tile([128, ng * BLOCK * D], mybir.dt.bfloat16, tag="vt")
            vap = vtile[:].rearrange("p (g n) -> p g n", g=ng)
            if mode != "compute":
                if mode != "dmav":
                    nc.gpsimd.dma_gather(
                        out_ap=kap,
                        in_ap=k_src[:],
                        idxs_ap=idx_sb[:, col:col + nk // 16],
                        num_idxs=nk,
                        num_idxs_reg=nb,
                        elem_size=BLOCK * D,
                        transpose=True,
                    )
                if mode != "dmak":
                    nc.gpsimd.dma_gather(
                        out_ap=vap,
                        in_ap=v_src[:],
                        idxs_ap=idx_sb[:, col:col + nk // 16],
                        num_idxs=nk,
                        num_idxs_reg=nb,
                        elem_size=BLOCK * D,
                        transpose=False,
                    )
            elif _rep == 0 and bi < max(kb, vb):
                nc.vector.memset(ktile[:], 0.0)
                nc.vector.memset(vtile[:], 0.0)
            col += nk // 16
            if mode.startswith("dma"):
                # minimal consumer so the gathers aren't dead: copy a sliver
                sliver = epool.tile([128, 4], mybir.dt.float32, tag="slv")
                if mode != "dmav":
                    nc.vector.tensor_copy(out=sliver[:], in_=kap[:, 0, 0:4])
                if mode != "dmak":
                    nc.vector.tensor_copy(out=sliver[:], in_=vap[:, 0, 0:4])
                if bi == B - 1:
                    fin0 = epool.tile([G, D], mybir.dt.float32, tag="fin")
                    nc.vector.memset(fin0[:], 0.0)
                    for bb in range(B):
                        nc.sync.dma_start(out=out_d[bb], in_=fin0[:])
                continue

            out_ps = opsum.tile([128, G], mybir.dt.float32, tag="ops")
            den_ps = dpsum.tile([16 * G, 1], mybir.dt.float32, tag="dps")

            # static valid-row count for chunk (g2, t)
            def jt_of(g2, t):
                jc = fb + (1 if t < rem else 0)
                return max(0, min(128, jc - 128 * g2))

            first_pv = True
            n_pv = sum(1 for g2 in range(ng) for t in range(16) if jt_of(g2, t) > 0)
            pv_i = 0
            for g2 in range(ng):
                scores_ps = spsum.tile([128, 16 * G], mybir.dt.float32, tag="sps")
                for t in range(16):
                    if jt_of(g2, t) == 0:
                        continue
                    nc.tensor.matmul(
                        scores_ps[:, 4 * t:4 * t + 4],
                        lhsT=kap[:, t, 128 * g2:128 * g2 + 128],
                        rhs=qT_b[:, 4 * b:4 * b + 4],
                        start=True, stop=True,
                    )
                # is any (row, col) of this page-group's p tile garbage?
                nbr = nb - 128 * g2
                partial = (g2 == ng - 1) and (nbr < 128 or rem > 0)
                ptile = ppool.tile([128, 16 * G], mybir.dt.bfloat16, tag="pt")
                if not partial:
                    nc.scalar.activation(ptile[:], scores_ps[:],
                                         mybir.ActivationFunctionType.Exp)
                else:
                    # exp into a temp, then keep only in-context entries so
                    # garbage (possibly NaN/inf) never reaches den/PV.
                    ptmp = ppool.tile([128, 16 * G], mybir.dt.bfloat16, tag="ptmp")
                    nc.scalar.activation(ptmp[:], scores_ps[:],
                                         mybir.ActivationFunctionType.Exp)
                    msk = epool.tile([128, 16 * G], mybir.dt.int8, tag="msk")
                    nc.sync.dma_start(out=msk[:], in_=mask_d[b])
                    nc.vector.memset(ptile[:], 0.0)
                    nc.vector.copy_predicated(ptile[:], msk[:], ptmp[:])
                # denominator contribution of this page-group
                nc.tensor.matmul(
                    den_ps[:],
                    lhsT=ptile[:],
                    rhs=ones_b[:],
                    start=(g2 == 0), stop=(g2 == ng - 1),
                )
                # PV accumulation
                for t in range(16):
                    jt = jt_of(g2, t)
                    if jt == 0:
                        continue
                    pv_i += 1
                    nc.tensor.matmul(
                        out_ps[:],
                        lhsT=vap[0:jt, g2, D * t:D * t + D],
                        rhs=ptile[0:jt, 4 * t:4 * t + 4],
                        start=first_pv, stop=(pv_i == n_pv),
                    )
                    first_pv = False

            # epilogue: out_ps [128,4] -> transpose -> scale by 1/den -> stage
            o_sb = epool.tile([128, G], mybir.dt.float32, tag="osb")
            nc.vector.tensor_copy(out=o_sb[:], in_=out_ps[:])
            oT_ps = tpsum.tile([G, 128], mybir.dt.float32, tag="otp")
            nc.tensor.transpose(oT_ps[:], o_sb[:], ident[:])

            den_sb = epool.tile([16 * G, 1], mybir.dt.float32, tag="dsb")
            nc.vector.tensor_copy(out=den_sb[:], in_=den_ps[:])
            den4_ps = npsum.tile([G, 1], mybir.dt.float32, tag="d4p")
            nc.tensor.matmul(den4_ps[:], lhsT=sel_sb[:], rhs=den_sb[:],
                             start=True, stop=True)
            den4_sb = epool.tile([G, 1], mybir.dt.float32, tag="d4s")
            nc.vector.tensor_copy(out=den4_sb[:], in_=den4_ps[:])
            rcp = epool.tile([G, 1], mybir.dt.float32, tag="rcp")
            nc.vector.reciprocal(rcp[:], den4_sb[:])

            fin = epool.tile([G, D], mybir.dt.float32, tag="fin")
            nc.vector.tensor_tensor(
                out=fin[:],
                in0=oT_ps[:],
                in1=rcp[:].to_broadcast([G, D]),
                op=mybir.AluOpType.mult,
            )
            nc.sync.dma_start(out=out_d[b], in_=fin[:])

    nc.compile()
    return nc


def _prep_host(q, k_cache, v_cache, block_tables, context_lens):
    """Shard + reformat inputs for the 8 cores. Returns in_maps list."""
    ctx_lens = np.asarray(context_lens, dtype=np.int64)
    bt = np.asarray(block_tables, dtype=np.int64)
    nbs = [max(1, -(-int(c) // BLOCK)) for c in ctx_lens]
    nks = [_round_up(nb, 128) for nb in nbs]
    idx_cols = sum(nk // 16 for nk in nks)

    # idx columns are packed in the same largest-first order the graph
    # builder iterates sequences in (see _build_graph).
    order = list(np.argsort(-np.asarray(nbs), kind="stable"))
    idx16 = np.full((16, idx_cols), -1, dtype=np.int16)
    col = 0
    for b in order:
        nb, nk = nbs[b], nks[b]
        ids = np.full(nk, -1, dtype=np.int16)
        ids[:nb] = bt[b, :nb].astype(np.int16)
        idx16[:, col:col + nk // 16] = ids.reshape(nk // 16, 16).T
        col += nk // 16
    idx_all = np.tile(idx16, (8, 1))  # replicate across the 8 Q7 cores

    sel = np.zeros((16 * G, G), dtype=np.float32)
    for i in range(16 * G):
        sel[i, i % G] = 1.0

    # validity mask of the LAST page-group of each sequence:
    # mask[b, j, 4t+g] = 1 iff token (page 128*(ng-1)+j, slot t) < ctx
    mask = np.zeros((B, 128, 16 * G), dtype=np.int8)
    jv = np.arange(128)
    tv = np.arange(BLOCK)
    for b in range(B):
        ctx_b = int(ctx_lens[b])
        g2 = nks[b] // 128 - 1
        pos = BLOCK * (128 * g2 + jv[:, None]) + tv[None, :]  # [128, 16]
        m = (pos < ctx_b).astype(np.int8)  # [128, 16]
        mask[b] = np.repeat(m, G, axis=1)

    q = np.asarray(q, dtype=np.float32)
    kc = np.asarray(k_cache, dtype=np.float32)
    vc = np.asarray(v_cache, dtype=np.float32)

    in_maps = []
    for c in range(KVH):
        k_shard = np.ascontiguousarray(kc[:, :, c, :]).astype(BF16).reshape(NBLK, BLOCK * D)
        v_shard = np.ascontiguousarray(vc[:, :, c, :]).astype(BF16).reshape(NBLK, BLOCK * D)
        qs = np.ascontiguousarray(q[:, G * c:G * c + G, :] * SCALE)  # [32,4,128]
        qT = np.ascontiguousarray(qs.reshape(B * G, D).T.astype(np.float32))
        in_maps.append({
            "k_src": k_shard,
            "v_src": v_shard,
            "qT": qT,
            "idx": idx_all,
            "sel": sel,
            "mask": mask,
        })
    return in_maps


def _get_graph(context_lens, repeat=1, bufs=(3, 3, 3, 2, 2, 2), mode="full"):
    key = (bytes(np.asarray(context_lens, dtype=np.int32)), repeat, bufs, mode)
    if key not in _GRAPH_CACHE:
        _GRAPH_CACHE[key] = _build_graph(
            np.asarray(context_lens, dtype=np.int64), repeat=repeat, bufs=bufs,
            mode=mode)
    return _GRAPH_CACHE[key]


def kernel_run(q, k_cache, v_cache, block_tables, context_lens, trace=False):
    """Run on the 8 NeuronCores; returns (out, BassKernelResults)."""
    import time
    from concourse.bass_utils import run_bass_kernel_spmd

    nc = _get_graph(context_lens)
    in_maps = _prep_host(q, k_cache, v_cache, block_tables, context_lens)
    last_exc = None
    for attempt in range(3):
        try:
            res = run_bass_kernel_spmd(nc, in_maps, core_ids=list(range(8)),
                                       trace=trace)
            break
        except Exception as e:  # transient device wedge (e.g. NRT_EXEC_UNIT_
            last_exc = e        # UNRECOVERABLE) — back off and retry
            time.sleep(5 * (attempt + 1))
    else:
        raise last_exc
    outs = [np.asarray(r["out"], dtype=np.float32) for r in res.results]
    out = np.concatenate(outs, axis=1).reshape(B, H, D)
    return out, res


def kernel(q, k_cache, v_cache, block_tables, context_lens):
    out, _ = kernel_run(q, k_cache, v_cache, block_tables, context_lens,
                        trace=False)
    return out



# revision 9
# speedup vs baseline: 1.9073x; 1.9073x over previous
"""Paged-attention decode kernel for 8 TRN2 NeuronCores (Bass/Tile).

Problem: nn_Attention_15229954031958 (sparse_attention, memory-bound).
  q [32, 32, 128] f32, k/v_cache [8192, 16, 8, 128] f32,
  block_tables [32, 256] i32, context_lens [32] i32 -> out [32, 32, 128] f32.

Sharding: tensor-parallel over KV heads. Core c holds the head-c slice of
both caches (cast to bf16) plus q heads 4c..4c+3; no cross-core
communication. Every core runs the same graph (trip counts baked from
context_lens, identical on all cores), so SPMD is trivially satisfied.

Device-side dataflow per sequence (nb = ceil(ctx/16) pages):
  - dma_gather(transpose=True) pulls the nb K pages straight into
    K^T layout [d=128, t=16, page] (bf16, 16-bit xbar transpose).
  - dma_gather(transpose=False) pulls V pages into [page, (t, d)].
  - per 128-token chunk (page-group g, token-slot t):
      scores[s,4] = ktile[:, t, g*128:...]^T @ qT[:, 4b:4b+4]  (PSUM)
  - one batched exp per page-group on ScalarE: [128, 64] PSUM -> bf16 p.
  - PV: out[d,4] += vtile[0:jt, g, t*128:...]^T @ p[0:jt, 4t:4t+4]
    accumulated in PSUM over all chunks; masking is done by the static
    contraction subrange jt (tokens beyond context_len are never read).
  - denominators: one matmul per page-group (p^T @ ones -> [64,1] PSUM,
    accumulated), collapsed to [4,1] with a selector matmul at the end.
  - epilogue: PE-transpose out to [4,128], multiply by 1/den, and DMA the
    [4,128] result slice per sequence.
"""

import os
import numpy as np
import ml_dtypes

BLOCK = 16
D = 128
B = 32
H = 32
KVH = 8
G = 4
NBLK = 8192
MAXB = 256
SCALE = 0.08838834764831845
BF16 = ml_dtypes.bfloat16
F8E3 = ml_dtypes.float8_e3m4

_GRAPH_CACHE = {}


def _round_up(x, m):
    return (x + m - 1) // m * m


def _build_graph(ctx_lens, repeat=1, bufs=(3, 3, 3, 2, 2, 2), mode="full",
                 kdt="fp8", vq=1):
    """Build + compile the SPMD graph for the given context lengths.

    repeat > 1 duplicates the whole body (for timing: slope difference
    between repeat=R and repeat=1 isolates pure HW time).
    mode: "full" | "dma" (gathers only) | "compute" (no gathers).
    kdt: "fp8" (e3m4 K cache, host-swizzled for the 16-bit xbar) | "bf16".
    vq: SWDGE queue for the V gather (K is always queue 0)."""
    import concourse.bass as bass
    import concourse.tile as tile
    from concourse import bacc, mybir
    from concourse.masks import make_identity
    from contextlib import ExitStack

    kb, vb, pb, eb, sb, ob = bufs
    nbs = [max(1, -(-int(c) // BLOCK)) for c in ctx_lens]
    nks = [_round_up(nb, 128) for nb in nbs]
    idx_cols = sum(nk // 16 for nk in nks)
    # process sequences largest-first: best prefetch ramp at the start and a
    # minimal non-overlapped tail (last gather is the smallest sequence)
    order = list(np.argsort(-np.asarray(nbs), kind="stable"))

    nc = bacc.Bacc("TRN2", target_bir_lowering=False, debug=False,
                   num_swdge_queues=(2 if vq else 1))

    kdtype = mybir.dt.float8e3 if kdt == "fp8" else mybir.dt.bfloat16
    k_src = nc.dram_tensor("k_src", [NBLK, BLOCK * D], kdtype,
                           kind="ExternalInput").ap()
    v_src = nc.dram_tensor("v_src", [NBLK, BLOCK * D], mybir.dt.bfloat16,
                           kind="ExternalInput").ap()
    qT_d = nc.dram_tensor("qT", [D, B * G], mybir.dt.float32,
                          kind="ExternalInput").ap()
    idx_d = nc.dram_tensor("idx", [128, idx_cols], mybir.dt.int16,
                           kind="ExternalInput").ap()
    sel_d = nc.dram_tensor("sel", [16 * G, G], mybir.dt.float32,
                           kind="ExternalInput").ap()
    mask_d = nc.dram_tensor("mask", [B, 128, 16 * G], mybir.dt.int8,
                            kind="ExternalInput").ap()
    out_d = nc.dram_tensor("out", [B, G, D], mybir.dt.float32,
                           kind="ExternalOutput").ap()

    with tile.TileContext(nc) as tc, ExitStack() as ctx:
        const = ctx.enter_context(tc.tile_pool(name="const", bufs=1))
        kpool = ctx.enter_context(tc.tile_pool(name="kpool", bufs=kb))
        vpool = ctx.enter_context(tc.tile_pool(name="vpool", bufs=vb))
        ppool = ctx.enter_context(tc.tile_pool(name="ppool", bufs=pb))
        epool = ctx.enter_context(tc.tile_pool(name="epool", bufs=eb))
        spsum = ctx.enter_context(tc.tile_pool(name="spsum", bufs=sb, space="PSUM"))
        opsum = ctx.enter_context(tc.tile_pool(name="opsum", bufs=ob, space="PSUM"))
        dpsum = ctx.enter_context(tc.tile_pool(name="dpsum", bufs=1, space="PSUM"))
        tpsum = ctx.enter_context(tc.tile_pool(name="tpsum", bufs=1, space="PSUM"))
        npsum = ctx.enter_context(tc.tile_pool(name="npsum", bufs=1, space="PSUM"))

        qT_sb = const.tile([128, B * G], mybir.dt.float32)
        nc.sync.dma_start(out=qT_sb[:], in_=qT_d[:])
        qT_b = const.tile([128, B * G], mybir.dt.bfloat16)
        nc.vector.tensor_copy(out=qT_b[:], in_=qT_sb[:])

        ident = const.tile([128, 128], mybir.dt.float32)
        make_identity(nc, ident[:])

        ones_b = const.tile([128, 1], mybir.dt.bfloat16)
        nc.vector.memset(ones_b[:], 1.0)

        sel_sb = const.tile([16 * G, G], mybir.dt.float32)
        nc.sync.dma_start(out=sel_sb[:], in_=sel_d[:])

        idx_sb = const.tile([128, idx_cols], mybir.dt.int16)
        nc.sync.dma_start(out=idx_sb[:], in_=idx_d[:])

        for _rep in range(repeat):
          col = 0
          for bi, b in enumerate(order):
            ctx_b = int(ctx_lens[b])
            nb, nk = nbs[b], nks[b]
            ng = nk // 128
            fb = ctx_b // BLOCK       # full pages
            rem = ctx_b % BLOCK       # tokens in the partial page

            ktile = kpool.tile([128, 16 * nk], kdtype, tag="kt")
            kap = ktile[:].rearrange("p (t n) -> p t n", t=16)
            if kdt == "fp8":
                # physical layout after the 16-bit-granularity transposed
                # gather of host-swizzled pages: [d=128, j=8, page, c=2]
                # where token t = 2j + c (see _prep_host).
                kap8 = ktile[:].rearrange("p (j n c) -> p j n c", j=8, c=2)
            vtile = vpool.tile([128, ng * BLOCK * D], mybir.dt.bfloat16, tag="vt")
            vap = vtile[:].rearrange("p (g n) -> p g n", g=ng)
            if mode != "compute":
                if mode != "dmav":
                    nc.gpsimd.dma_gather(
                        out_ap=kap,
                        in_ap=k_src[:],
                        idxs_ap=idx_sb[:, col:col + nk // 16],
                        num_idxs=nk,
                        num_idxs_reg=nb,
                        elem_size=BLOCK * D,
                        transpose=True,
                    )
                if mode != "dmak":
                    nc.gpsimd.dma_gather(
                        out_ap=vap,
                        in_ap=v_src[:],
                        idxs_ap=idx_sb[:, col:col + nk // 16],
                        num_idxs=nk,
                        num_idxs_reg=nb,
                        elem_size=BLOCK * D,
                        transpose=False,
                        queue_num=vq,
                    )
            elif _rep == 0 and bi < max(kb, vb):
                nc.vector.memset(ktile[:], 0.0)
                nc.vector.memset(vtile[:], 0.0)
            col += nk // 16
            if mode.startswith("dma"):
                # minimal consumer so the gathers aren't dead: copy a sliver
                sliver = epool.tile([128, 4], mybir.dt.float32, tag="slv")
                if mode != "dmav":
                    nc.vector.tensor_copy(out=sliver[:], in_=kap[:, 0, 0:4])
                if mode != "dmak":
                    nc.vector.tensor_copy(out=sliver[:], in_=vap[:, 0, 0:4])
                if bi == B - 1:
                    fin0 = epool.tile([G, D], mybir.dt.float32, tag="fin")
                    nc.vector.memset(fin0[:], 0.0)
                    for bb in range(B):
                        nc.sync.dma_start(out=out_d[bb], in_=fin0[:])
                continue

            out_ps = opsum.tile([128, G], mybir.dt.float32, tag="ops")
            den_ps = dpsum.tile([16 * G, 1], mybir.dt.float32, tag="dps")

            # static valid-row count for chunk (g2, t)
            def jt_of(g2, t):
                jc = fb + (1 if t < rem else 0)
                return max(0, min(128, jc - 128 * g2))

            first_pv = True
            n_pv = sum(1 for g2 in range(ng) for t in range(16) if jt_of(g2, t) > 0)
            pv_i = 0
            for g2 in range(ng):
                scores_ps = spsum.tile([128, 16 * G], mybir.dt.float32, tag="sps")
                for t in range(16):
                    if jt_of(g2, t) == 0:
                        continue
                    if kdt == "fp8":
                        klhsT = kap8[:, t // 2, 128 * g2:128 * g2 + 128, t % 2]
                    else:
                        klhsT = kap[:, t, 128 * g2:128 * g2 + 128]
                    nc.tensor.matmul(
                        scores_ps[:, 4 * t:4 * t + 4],
                        lhsT=klhsT,
                        rhs=qT_b[:, 4 * b:4 * b + 4],
                        start=True, stop=True,
                    )
                # is any (row, col) of this page-group's p tile garbage?
                nbr = nb - 128 * g2
                partial = (g2 == ng - 1) and (nbr < 128 or rem > 0)
                ptile = ppool.tile([128, 16 * G], mybir.dt.bfloat16, tag="pt")
                if not partial:
                    nc.scalar.activation(ptile[:], scores_ps[:],
                                         mybir.ActivationFunctionType.Exp)
                else:
                    # exp into a temp, then keep only in-context entries so
                    # garbage (possibly NaN/inf) never reaches den/PV.
                    ptmp = ppool.tile([128, 16 * G], mybir.dt.bfloat16, tag="ptmp")
                    nc.scalar.activation(ptmp[:], scores_ps[:],
                                         mybir.ActivationFunctionType.Exp)
                    msk = epool.tile([128, 16 * G], mybir.dt.int8, tag="msk")
                    nc.sync.dma_start(out=msk[:], in_=mask_d[b])
                    nc.vector.memset(ptile[:], 0.0)
                    nc.vector.copy_predicated(ptile[:], msk[:], ptmp[:])
                # denominator contribution of this page-group
                nc.tensor.matmul(
                    den_ps[:],
                    lhsT=ptile[:],
                    rhs=ones_b[:],
                    start=(g2 == 0), stop=(g2 == ng - 1),
                )
                # PV accumulation
                for t in range(16):
                    jt = jt_of(g2, t)
                    if jt == 0:
                        continue
                    pv_i += 1
                    nc.tensor.matmul(
                        out_ps[:],
                        lhsT=vap[0:jt, g2, D * t:D * t + D],
                        rhs=ptile[0:jt, 4 * t:4 * t + 4],
                        start=first_pv, stop=(pv_i == n_pv),
                    )
                    first_pv = False

            # epilogue: out_ps [128,4] -> transpose -> scale by 1/den -> stage
            o_sb = epool.tile([128, G], mybir.dt.float32, tag="osb")
            nc.vector.tensor_copy(out=o_sb[:], in_=out_ps[:])
            oT_ps = tpsum.tile([G, 128], mybir.dt.float32, tag="otp")
            nc.tensor.transpose(oT_ps[:], o_sb[:], ident[:])

            den_sb = epool.tile([16 * G, 1], mybir.dt.float32, tag="dsb")
            nc.vector.tensor_copy(out=den_sb[:], in_=den_ps[:])
            den4_ps = npsum.tile([G, 1], mybir.dt.float32, tag="d4p")
            nc.tensor.matmul(den4_ps[:], lhsT=sel_sb[:], rhs=den_sb[:],
                             start=True, stop=True)
            den4_sb = epool.tile([G, 1], mybir.dt.float32, tag="d4s")
            nc.vector.tensor_copy(out=den4_sb[:], in_=den4_ps[:])
            rcp = epool.tile([G, 1], mybir.dt.float32, tag="rcp")
            nc.vector.reciprocal(rcp[:], den4_sb[:])

            fin = epool.tile([G, D], mybir.dt.float32, tag="fin")
            nc.vector.tensor_tensor(
                out=fin[:],
                in0=oT_ps[:],
                in1=rcp[:].to_broadcast([G, D]),
                op=mybir.AluOpType.mult,
            )
            nc.sync.dma_start(out=out_d[b], in_=fin[:])

    nc.compile()
    return nc


def _prep_host(q, k_cache, v_cache, block_tables, context_lens, kdt="fp8"):
    """Shard + reformat inputs for the 8 cores. Returns in_maps list."""
    ctx_lens = np.asarray(context_lens, dtype=np.int64)
    bt = np.asarray(block_tables, dtype=np.int64)
    nbs = [max(1, -(-int(c) // BLOCK)) for c in ctx_lens]
    nks = [_round_up(nb, 128) for nb in nbs]
    idx_cols = sum(nk // 16 for nk in nks)

    # idx columns are packed in the same largest-first order the graph
    # builder iterates sequences in (see _build_graph).
    order = list(np.argsort(-np.asarray(nbs), kind="stable"))
    idx16 = np.full((16, idx_cols), -1, dtype=np.int16)
    col = 0
    for b in order:
        nb, nk = nbs[b], nks[b]
        ids = np.full(nk, -1, dtype=np.int16)
        ids[:nb] = bt[b, :nb].astype(np.int16)
        idx16[:, col:col + nk // 16] = ids.reshape(nk // 16, 16).T
        col += nk // 16
    idx_all = np.tile(idx16, (8, 1))  # replicate across the 8 Q7 cores

    sel = np.zeros((16 * G, G), dtype=np.float32)
    for i in range(16 * G):
        sel[i, i % G] = 1.0

    # validity mask of the LAST page-group of each sequence:
    # mask[b, j, 4t+g] = 1 iff token (page 128*(ng-1)+j, slot t) < ctx
    mask = np.zeros((B, 128, 16 * G), dtype=np.int8)
    jv = np.arange(128)
    tv = np.arange(BLOCK)
    for b in range(B):
        ctx_b = int(ctx_lens[b])
        g2 = nks[b] // 128 - 1
        pos = BLOCK * (128 * g2 + jv[:, None]) + tv[None, :]  # [128, 16]
        m = (pos < ctx_b).astype(np.int8)  # [128, 16]
        mask[b] = np.repeat(m, G, axis=1)

    q = np.asarray(q, dtype=np.float32)
    kc = np.asarray(k_cache, dtype=np.float32)
    vc = np.asarray(v_cache, dtype=np.float32)

    in_maps = []
    for c in range(KVH):
        ks = np.ascontiguousarray(kc[:, :, c, :])  # [NBLK, 16, 128] f32
        if kdt == "fp8":
            # page layout (t, d) -> (t//2, d, t%2) so the 16-bit-granularity
            # transposed gather lands K^T as [d, j, page, t%2]
            k_shard = np.ascontiguousarray(
                ks.reshape(NBLK, 8, 2, 128).transpose(0, 1, 3, 2)
            ).astype(F8E3).reshape(NBLK, BLOCK * D)
        else:
            k_shard = ks.astype(BF16).reshape(NBLK, BLOCK * D)
        v_shard = np.ascontiguousarray(vc[:, :, c, :]).astype(BF16).reshape(NBLK, BLOCK * D)
        qs = np.ascontiguousarray(q[:, G * c:G * c + G, :] * SCALE)  # [32,4,128]
        qT = np.ascontiguousarray(qs.reshape(B * G, D).T.astype(np.float32))
        in_maps.append({
            "k_src": k_shard,
            "v_src": v_shard,
            "qT": qT,
            "idx": idx_all,
            "sel": sel,
            "mask": mask,
        })
    return in_maps


def _get_graph(context_lens, repeat=1, bufs=(3, 3, 3, 2, 2, 2), mode="full",
               kdt="fp8", vq=1):
    key = (bytes(np.asarray(context_lens, dtype=np.int32)), repeat, bufs, mode,
           kdt, vq)
    if key not in _GRAPH_CACHE:
        _GRAPH_CACHE[key] = _build_graph(
            np.asarray(context_lens, dtype=np.int64), repeat=repeat, bufs=bufs,
            mode=mode, kdt=kdt, vq=vq)
    return _GRAPH_CACHE[key]


def kernel_run(q, k_cache, v_cache, block_tables, context_lens, trace=False):
    """Run on the 8 NeuronCores; returns (out, BassKernelResults)."""
    import time
    from concourse.bass_utils import run_bass_kernel_spmd

    nc = _get_graph(context_lens)
    in_maps = _prep_host(q, k_cache, v_cache, block_tables, context_lens)
    last_exc = None
    for attempt in range(3):
        try:
            res = run_bass_kernel_spmd(nc, in_maps, core_ids=list(range(8)),
                                       trace=trace)
            break
        except Exception as e:  # transient device wedge (e.g. NRT_EXEC_UNIT_
            last_exc = e        # UNRECOVERABLE) — back off and retry
            time.sleep(5 * (attempt + 1))
    else:
        raise last_exc
    outs = [np.asarray(r["out"], dtype=np.float32) for r in res.results]
    out = np.concatenate(outs, axis=1).reshape(B, H, D)
    return out, res


def kernel(q, k_cache, v_cache, block_tables, context_lens):
    out, _ = kernel_run(q, k_cache, v_cache, block_tables, context_lens,
                        trace=False)
    return out

